# revision 1
# baseline (speedup 1.0000x reference)
"""Trainium2 Bass kernel for nn_CrossAttention_72275709657317.

Reference computation (B=4, S=2048, E=1024, D=64):
    Q = x @ Wq.T + bq                      [B,S,D]
    K = y @ Wk.T + bk                      [B,S,D]
    scores = Q @ K.T / sqrt(D)             [B,Sq,Sk]
    attn = softmax(scores, axis=1)         (softmax over the QUERY axis)
    V = (y @ WvR.T + bvR) @ WvL.T + bvL    [B,S,E]
    out = attn @ V                         [B,S,E]

Key algebraic restructuring:
  * V is rank-64 (+bias), so attn @ V = (attn @ [VR | 1]) @ [[WvL.T],[bvL]]
    -- the dominant S*S*E matmul collapses to S*S*D.
  * softmax over q: attn[q,k] = exp(s[q,k])/den[k], den[k] = sum_q exp(s[q,k]).
    den only enters per-k, so it is folded into the VR' rows; attnT itself is
    kept unnormalized.

Sharding: 8 cores -> (batch b = c//2, query-half h = c%2). Each core computes
K/VR projections for its local k-half; the pair exchanges them (and the exp
column-sum partials) via pairwise AllReduce.  All cross-core data uses the
"partner = pair_sum - mine" identity so the single SPMD program is h-agnostic.

Matmuls run in fp32r (full PE speed, ~1.5e-4 rel err).  HW quirk: fp32r
ACCUMULATING chains require the full 128-wide lhsT free dim (M<128 chains hang
the exec unit), so K/VR projections are fused into one M=128 chain (psum rows
0:64 = K^T, 64:128 = VR^T), the Q chain duplicates Wq, and VR' is zero-padded
to M=128 for the O1 chain.
"""
import numpy as np

import concourse.bass as bass
import concourse.tile as tile
from concourse import bacc, mybir
from concourse.masks import make_identity
from concourse.bass_utils import run_bass_kernel_spmd

N_CORES = 8
B, S, E, D = 4, 2048, 1024, 64
H = S // 2            # per-core q rows / local k rows
P = 128
EB = E // P           # 8 e-chunks
BLK = 256             # s-rows per transpose/projection block
BCH = BLK // P        # 2
NBLK = H // BLK       # 4
KC = S // P           # 16 k-chunks
KCL = H // P          # 8 local k-chunks
NQ = H // 512         # 2 q-chunks of 512
DV = D + 1            # VR plus folded-ones column
F32 = mybir.dt.float32
F32R = mybir.dt.float32r
EXP = mybir.ActivationFunctionType.Exp
ADD = mybir.AluOpType.add
GROUPS = [[0, 1], [2, 3], [4, 5], [6, 7]]

IN_SPECS = [
    ("x", [H, E]), ("y", [H, E]),
    ("Wq", [D, E]), ("bq", [D]), ("Wk", [D, E]), ("bk", [D]),
    ("WvR", [D, E]), ("bvR", [D]), ("WvL", [E, D]), ("bvL", [E]),
]


def _emit(tc, aps, out_ap, no_cc=False, no_accum=False, stop_stage=99):
    nc = tc.nc
    from contextlib import ExitStack
    with ExitStack() as ctx:
        const = ctx.enter_context(tc.tile_pool(name="const", bufs=1))
        io = ctx.enter_context(tc.tile_pool(name="io", bufs=3))
        tb = ctx.enter_context(tc.tile_pool(name="tb", bufs=2))
        work = ctx.enter_context(tc.tile_pool(name="work", bufs=2))
        big = ctx.enter_context(tc.tile_pool(name="big", bufs=1))
        tp_ps = ctx.enter_context(tc.tile_pool(name="tp_ps", bufs=3, space="PSUM"))
        mm_ps = ctx.enter_context(tc.tile_pool(name="mm_ps", bufs=2, space="PSUM"))
        o1_ps = ctx.enter_context(tc.tile_pool(name="o1_ps", bufs=2, space="PSUM"))
        dram = ctx.enter_context(tc.tile_pool(name="dram", bufs=1, space="DRAM"))

        # ---------------- constants ----------------
        ident = const.tile([P, P], F32)
        make_identity(nc, ident[:])
        zeros64 = const.tile([P, D], F32)
        nc.gpsimd.memset(zeros64[:], 0.0)

        if stop_stage <= -3:
            _early = const.tile([P, P], F32, name="early")
            nc.vector.tensor_copy(_early[:], ident[:])
            nc.sync.dma_start(out_ap[0:P, 0:P], _early[:])
            return

        # biases: contiguous [1,64] rows -> one PE transpose -> per-partition cols
        bias_stage = const.tile([P, D], F32)
        nc.sync.dma_start(bias_stage[0:1, :], aps["bk"].rearrange("(o f) -> o f", o=1))
        nc.sync.dma_start(bias_stage[1:2, :], aps["bvR"].rearrange("(o f) -> o f", o=1))
        nc.sync.dma_start(bias_stage[2:3, :], aps["bq"].rearrange("(o f) -> o f", o=1))
        bias_ps = tp_ps.tile([P, 4 * P], F32, name="tp4")
        nc.tensor.transpose(bias_ps[0:D, 0:P], bias_stage[:], ident[:])
        bias_kv = const.tile([P, 1], F32)
        nc.vector.tensor_copy(bias_kv[0:D, :], bias_ps[0:D, 0:1])
        nc.vector.tensor_copy(bias_kv[D:P, :], bias_ps[0:D, 1:2])
        bias_q = const.tile([D, 1], F32)
        nc.vector.tensor_copy(bias_q[:], bias_ps[0:D, 2:3])

        if stop_stage <= -2:
            _early = const.tile([P, 1], F32, name="early2")
            nc.vector.tensor_copy(_early[:], bias_kv[:])
            nc.sync.dma_start(out_ap[0:P, 0:1], _early[:])
            return

        # fused lhsT weights: WkvT[:, ei, 0:64] = Wk^T, [:, ei, 64:128] = WvR^T
        # WqqT duplicates Wq^T into both halves (fp32r chains need M=128).
        def build_fused_wT(name, src_lo, src_hi):
            wt = const.tile([P, EB, P], F32R, name=name)
            for half, src in ((0, src_lo), (1, src_hi)):
                stage = const.tile([P, E], F32, name=f"stage_{name}_{half}")
                nc.gpsimd.memset(stage[:], 0.0)
                nc.sync.dma_start(stage[0:D, :], aps[src])
                for g in range(2):
                    ps = tp_ps.tile([P, 4 * P], F32, name="tp4")
                    for j in range(4):
                        ei = 4 * g + j
                        nc.tensor.transpose(ps[:, j * P:(j + 1) * P],
                                            stage[:, ei * P:(ei + 1) * P], ident[:])
                    nc.vector.tensor_copy(
                        wt[:, 4 * g:4 * g + 4, half * D:half * D + D],
                        ps[:].rearrange("p (a b) -> p a b", a=4)[:, :, 0:D])
            return wt

        WkvT = build_fused_wT("WkvT", "Wk", "WvR")
        WqqT = build_fused_wT("WqqT", "Wq", "Wq")

        if stop_stage <= -1:
            nc.sync.dma_start(out_ap[0:P, 0:P], WqqT[:, 0, :].bitcast(F32))
            return

        # WvLT: [DV, E] fp32r; row D = bvL
        wvls = const.tile([P, EB, D], F32)
        for vo in range(EB):
            nc.sync.dma_start(wvls[:, vo, :], aps["WvL"][vo * P:(vo + 1) * P, :])
        WvLT = const.tile([DV, E], F32R)
        for g in range(2):
            ps = tp_ps.tile([P, 4 * P], F32, name="tp4")
            for j in range(4):
                vo = 4 * g + j
                nc.tensor.transpose(ps[0:D, j * P:(j + 1) * P], wvls[:, vo, :], ident[:])
            nc.vector.tensor_copy(WvLT[0:D, g * 512:(g + 1) * 512], ps[0:D, :])
        bvls = const.tile([1, E], F32)
        nc.sync.dma_start(bvls[:], aps["bvL"].rearrange("(o f) -> o f", o=1))
        nc.vector.tensor_copy(WvLT[D:DV, :], bvls[:])

        # ---------------- persistent tiles ----------------
        KT = big.tile([D, S], F32R, name="KT")         # [64, 2048] scores lhsT
        QT = big.tile([D, H], F32R, name="QT")         # [64, 1024] scores rhs
        KTVR_l = big.tile([P, H], F32, name="KTVR_l")  # rows 0:64 K^T, 64:128 VR^T
        attnT = big.tile([P, KC, H], F32R, name="attnT")
        den2 = big.tile([P, KC, NQ], F32, name="den2")

        kv_loc = dram.tile([P, H], F32)
        kv_sum = dram.tile([P, H], F32)
        den_dram = dram.tile([P, KC], F32)
        den_sum_dram = dram.tile([P, KC], F32)

        def _dump_and_stop(tile_ap, rows, cols):
            nc.sync.dma_start(out_ap[0:rows, 0:cols], tile_ap)

        if stop_stage <= 0:
            _dump_and_stop(ident[:], P, P)
            return

        # ---------------- projection block pipeline ----------------
        def proj_blocks(src_ap, wt, bias, dst_fn, dst_rows):
            for blk in range(NBLK):
                xb = io.tile([P, BCH, E], F32, name="inblk")
                nc.sync.dma_start(
                    xb[:],
                    src_ap[blk * BLK:(blk + 1) * BLK, :]
                    .rearrange("(c p) e -> p c e", p=P))
                xT = tb.tile([P, EB, BLK], F32R, name="tblk")
                for c in range(BCH):
                    for g in range(2):
                        ps = tp_ps.tile([P, 4 * P], F32, name="tp4")
                        for j in range(4):
                            ei = 4 * g + j
                            nc.tensor.transpose(ps[:, j * P:(j + 1) * P],
                                                xb[:, c, ei * P:(ei + 1) * P],
                                                ident[:])
                        nc.vector.tensor_copy(
                            xT[:, 4 * g:4 * g + 4, c * P:(c + 1) * P],
                            ps[:].rearrange("p (a b) -> p a b", a=4))
                ps = mm_ps.tile([P, 512], F32, name="mmps")
                for ei in range(EB):
                    nc.tensor.matmul(ps[:, 0:BLK], wt[:, ei, :], xT[:, ei, :],
                                     start=(ei == 0), stop=(ei == EB - 1))
                nc.scalar.add(dst_fn(blk), ps[0:dst_rows, 0:BLK], bias[:])

        # y and x paths interleaved: earlier QT availability for local scores
        proj_blocks(aps["y"], WkvT, bias_kv,
                    lambda blk: KTVR_l[:, blk * BLK:(blk + 1) * BLK], P)

        if stop_stage <= 1:
            _dump_and_stop(KTVR_l[0:D, :], D, H)
            return

        # collective 1: exchange K^T / VR^T within the pair
        nc.sync.dma_start(kv_loc[:], KTVR_l[:])
        if no_cc:
            nc.sync.dma_start(kv_sum[:], kv_loc[:])
        else:
            nc.gpsimd.collective_compute(
                "AllReduce", ADD, replica_groups=GROUPS,
                ins=[kv_loc.opt()], outs=[kv_sum.opt()])
        kvs = big.tile([P, H], F32, name="kvs")
        nc.sync.dma_start(kvs[:], kv_sum[:])
        KTVR_r = big.tile([P, H], F32, name="KTVR_r")
        nc.vector.tensor_sub(KTVR_r[:], kvs[:], KTVR_l[:])   # partner = sum - mine
        nc.vector.tensor_copy(KT[:, 0:H], KTVR_l[0:D, :])    # rounded to fp32r
        nc.vector.tensor_copy(KT[:, H:S], KTVR_r[0:D, :])

        if stop_stage <= 2:
            _dump_and_stop(KT[:, 0:H].bitcast(F32), D, H)
            return

        # x path: Q^T (overlaps collective 1)
        proj_blocks(aps["x"], WqqT, bias_q,
                    lambda blk: QT[:, blk * BLK:(blk + 1) * BLK], D)

        if stop_stage <= 3:
            _dump_and_stop(QT[:].bitcast(F32), D, H)
            return

        # ---------------- scoresT + exp + den partials ----------------
        for kc in range(KC):
            for qc in range(NQ):
                sps = mm_ps.tile([P, 512], F32, name="mmps")
                nc.tensor.matmul(sps[:], KT[:, kc * P:(kc + 1) * P],
                                 QT[:, qc * 512:(qc + 1) * 512],
                                 start=True, stop=True)
                nc.scalar.activation(attnT[:, kc, qc * 512:(qc + 1) * 512], sps[:],
                                     EXP, scale=0.125,
                                     accum_out=None if no_accum else den2[:, kc, qc:qc + 1])

        if stop_stage <= 4:
            _dump_and_stop(attnT[:, 0, :].bitcast(F32), P, H)
            return

        # ---------------- VR unscaled transposes (overlap exp/den) ----------
        VRu = big.tile([P, KC, D], F32, name="VRu")
        VRp = big.tile([P, KC, P], F32R, name="VRp")
        for g in range(KC // 4):
            ps = tp_ps.tile([P, 4 * P], F32, name="tp4")
            for j in range(4):
                kc = 4 * g + j
                src_t = KTVR_l if kc < KCL else KTVR_r
                col = (kc if kc < KCL else kc - KCL) * P
                nc.tensor.transpose(ps[:, j * P:(j + 1) * P],
                                    src_t[:, col:col + P], ident[:])
            for j in range(4):
                kc = 4 * g + j
                nc.vector.tensor_copy(VRu[:, kc, :], ps[:, j * P + D:(j + 1) * P])
                nc.vector.tensor_copy(VRp[:, kc, DV:P], zeros64[:, 0:P - DV])

        # ---------------- den exchange + reciprocal ----------------
        den_loc = big.tile([P, KC], F32, name="den_loc")
        if no_accum:
            for kc in range(KC):
                nc.vector.tensor_reduce(den_loc[:, kc:kc + 1],
                                        attnT[:, kc, :].bitcast(F32),
                                        axis=mybir.AxisListType.X, op=ADD)
        else:
            nc.vector.tensor_reduce(den_loc[:], den2[:], axis=mybir.AxisListType.X, op=ADD)
        nc.sync.dma_start(den_dram[:], den_loc[:])
        if no_cc:
            nc.sync.dma_start(den_sum_dram[:], den_dram[:])
        else:
            nc.gpsimd.collective_compute(
                "AllReduce", ADD, replica_groups=GROUPS,
                ins=[den_dram.opt()], outs=[den_sum_dram.opt()])
        dsum = big.tile([P, KC], F32, name="dsum")
        nc.sync.dma_start(dsum[:], den_sum_dram[:])
        partner = big.tile([P, KC], F32, name="partner")
        nc.vector.tensor_sub(partner[:], dsum[:], den_loc[:])
        denf = big.tile([P, KC], F32, name="denf")
        # my chunk order is [local | remote]; partner's is swapped
        nc.vector.tensor_add(denf[:, 0:KCL], den_loc[:, 0:KCL], partner[:, KCL:KC])
        nc.vector.tensor_add(denf[:, KCL:KC], den_loc[:, KCL:KC], partner[:, 0:KCL])
        r_sb = big.tile([P, KC], F32, name="r_sb")
        nc.vector.reciprocal(r_sb[:], denf[:])

        if stop_stage <= 5:
            _dump_and_stop(r_sb[:], P, KC)
            return

        # ---------------- VR' = [VR * r | r | 0-pad] ----------------
        for kc in range(KC):
            nc.vector.tensor_scalar_mul(VRp[:, kc, 0:D], VRu[:, kc, :],
                                        r_sb[:, kc:kc + 1])
            nc.vector.tensor_copy(VRp[:, kc, D:DV], r_sb[:, kc:kc + 1])

        if stop_stage <= 6:
            _dump_and_stop(VRp[:, 0, :].bitcast(F32), P, P)
            return

        # ---------------- O1T = VR'^T @ attnT ----------------
        O1T = big.tile([DV, H], F32R, name="O1T")
        for qc in range(NQ):
            ops_ = o1_ps.tile([P, 512], F32, name="o1ps")
            for kc in range(KC):
                nc.tensor.matmul(ops_[:], VRp[:, kc, :],
                                 attnT[:, kc, qc * 512:(qc + 1) * 512],
                                 start=(kc == 0), stop=(kc == KC - 1))
            nc.scalar.copy(O1T[:, qc * 512:(qc + 1) * 512], ops_[0:DV, :])

        if stop_stage <= 7:
            _dump_and_stop(O1T[:].bitcast(F32), DV, H)
            return

        # ---------------- out = O1T^T @ WvL'T ----------------
        for qo in range(H // P):
            ot = work.tile([P, E], F32, name="outt")
            for vc in range(2):
                fps = mm_ps.tile([P, 512], F32, name="mmps")
                nc.tensor.matmul(fps[:], O1T[:, qo * P:(qo + 1) * P],
                                 WvLT[:, vc * 512:(vc + 1) * 512],
                                 start=True, stop=True)
                nc.vector.tensor_copy(ot[:, vc * 512:(vc + 1) * 512], fps[:])
            nc.sync.dma_start(out_ap[qo * P:(qo + 1) * P, :], ot[:])


def build_nc(reps: int = 1, no_cc=False, no_accum=False, stop_stage=99):
    nc = bacc.Bacc("TRN2", target_bir_lowering=False, debug=False,
                   num_devices=N_CORES)
    aps = {name: nc.dram_tensor(name, shape, F32, kind="ExternalInput").ap()
           for name, shape in IN_SPECS}
    out_ap = nc.dram_tensor("out", [H, E], F32, kind="ExternalOutput").ap()
    with tile.TileContext(nc) as tc:
        if reps == 1:
            _emit(tc, aps, out_ap, no_cc=no_cc, no_accum=no_accum, stop_stage=stop_stage)
        else:
            with tc.For_i(0, reps, 1):
                _emit(tc, aps, out_ap, no_cc=no_cc, no_accum=no_accum, stop_stage=stop_stage)
    nc.compile()
    return nc


def make_in_maps(inputs):
    arrs = {k: np.ascontiguousarray(np.asarray(v, dtype=np.float32))
            for k, v in inputs.items()}
    in_maps = []
    for c in range(N_CORES):
        b, h = divmod(c, 2)
        m = {"x": np.ascontiguousarray(arrs["x"][b, h * H:(h + 1) * H, :]),
             "y": np.ascontiguousarray(arrs["y"][b, h * H:(h + 1) * H, :])}
        for wn in ("Wq", "bq", "Wk", "bk", "WvR", "bvR", "WvL", "bvL"):
            m[wn] = arrs[wn]
        in_maps.append(m)
    return in_maps


def assemble_out(results):
    out = np.empty((B, S, E), dtype=np.float32)
    for c in range(N_CORES):
        b, h = divmod(c, 2)
        out[b, h * H:(h + 1) * H, :] = results[c]["out"]
    return out


_NC = None


def kernel(**inputs) -> np.ndarray:
    global _NC
    if _NC is None:
        _NC = build_nc()
    in_maps = make_in_maps(inputs)
    res = run_bass_kernel_spmd(_NC, in_maps, list(range(N_CORES)))
    return assemble_out(res.results)



# revision 27
# speedup vs baseline: 1.5543x; 1.5543x over previous
"""Trainium2 Bass kernel for nn_CrossAttention_72275709657317  (v2, bf16).

Reference computation (B=4, S=2048, E=1024, D=64):
    Q = x @ Wq.T + bq                      [B,S,D]
    K = y @ Wk.T + bk                      [B,S,D]
    scores = Q @ K.T / sqrt(D)             [B,Sq,Sk]
    attn = softmax(scores, axis=1)         (softmax over the QUERY axis)
    V = (y @ WvR.T + bvR) @ WvL.T + bvL    [B,S,E]
    out = attn @ V                         [B,S,E]

Restructuring:
  * V is rank-64: attn @ V = (attn @ [VR | 1]) @ [[WvL.T],[u]] with
    u = bvL + WvL @ bvR  (both V-path biases folded into one extra row).
  * softmax over q: attn[q,k] = e[q,k]/den[k], den[k] = sum_q e[q,k];
    1/den folded into the VR' rows, attnT kept unnormalized.
  * All matmul operands are bf16 (inputs/weights cast host-side; output
    returned bf16 and upcast host-side).  PSUM accumulation stays f32,
    as do the pairwise exchange (exact partner = pairsum - mine) and den.

Sharding: 8 cores -> (batch b = c//2, query-half h = c%2).  Each core
projects K/VR for its local k-half; the pair exchanges them (and den
partials) via pairwise f32 AllReduce with the sum-minus-mine identity,
so the single SPMD program is h-agnostic.
"""
import numpy as np

import concourse.bass as bass
import concourse.tile as tile
from concourse import bacc, mybir
from concourse.masks import make_identity
from concourse.bass_utils import run_bass_kernel_spmd

N_CORES = 8
B, S, E, D = 4, 2048, 1024, 64
H = S // 2            # per-core q rows / local k rows
P = 128
EB = E // P           # 8 e-chunks
NBLK = 4              # input blocks of 256 rows
KCL = 8               # local k-chunks of 128
KC = 16               # global k-chunks
DV = D + 1            # VR width plus folded-ones column
F32 = mybir.dt.float32
BF = mybir.dt.bfloat16
EXP = mybir.ActivationFunctionType.Exp
ADD = mybir.AluOpType.add
GROUPS = [[0, 1], [2, 3], [4, 5], [6, 7]]

IN_SPECS = [
    ("x", [H, E], BF), ("y", [H, E], BF),
    ("Wq", [D, E], BF), ("Wk", [D, E], BF), ("WvR", [D, E], BF),
    ("WvL", [E, D], BF),
    ("bq", [D], F32), ("bk", [D], F32), ("bvR", [D], F32), ("bvL", [E], F32),
]


def _emit(tc, aps, out_ap, no_cc=False, stop_stage=99):
    nc = tc.nc
    from contextlib import ExitStack
    with ExitStack() as ctx:
        const = ctx.enter_context(tc.tile_pool(name="const", bufs=1))
        io = ctx.enter_context(tc.tile_pool(name="io", bufs=4))
        big = ctx.enter_context(tc.tile_pool(name="big", bufs=1))
        outp = ctx.enter_context(tc.tile_pool(name="outp", bufs=3))
        dram = ctx.enter_context(tc.tile_pool(name="dram", bufs=1, space="DRAM"))

        # ---------------- constants / weights ----------------
        identB = const.tile([P, P], BF)
        make_identity(nc, identB[:])
        identF = const.tile([P, P], F32)
        make_identity(nc, identF[:])

        # ACT engine does zero DMAs: weights/biases/staging all ride the
        # gpsimd SWDGE queue (Pool otherwise idle); inputs own the SP queue
        Wk_s = const.tile([D, E], BF)
        nc.gpsimd.dma_start(Wk_s[:], aps["Wk"])
        WvR_s = const.tile([D, E], BF)
        nc.gpsimd.dma_start(WvR_s[:], aps["WvR"])
        Wq_s = const.tile([D, E], BF)
        nc.gpsimd.dma_start(Wq_s[:], aps["Wq"])
        wvls = const.tile([P, EB, D], BF)
        nc.gpsimd.dma_start(wvls[:], aps["WvL"].rearrange("(c p) d -> p c d", p=P))
        bias_stage = const.tile([4, D], F32)
        nc.gpsimd.dma_start(bias_stage[0:1, :], aps["bq"].rearrange("(o f) -> o f", o=1))
        nc.gpsimd.dma_start(bias_stage[1:2, :], aps["bk"].rearrange("(o f) -> o f", o=1))
        nc.gpsimd.dma_start(bias_stage[2:3, :], aps["bvR"].rearrange("(o f) -> o f", o=1))
        bvl_s = const.tile([1, E], F32)

        # input loads, SP queue, in stream order
        inb = []
        for src, i in [("x", 0), ("x", 1), ("y", 0), ("y", 1),
                       ("y", 2), ("y", 3), ("x", 2), ("x", 3)]:
            t = io.tile([P, 2, E], BF, name="inb")
            nc.sync.dma_start(
                t[:], aps[src][i * 256:(i + 1) * 256, :]
                .rearrange("(c p) e -> p c e", p=P))
            inb.append((src, i, t))
        inb = {(s, i): t for s, i, t in inb}

        # persistent tiles
        xT = big.tile([P, EB, H], BF, name="xT")
        yT = big.tile([P, EB, H], BF, name="yT")
        QT = big.tile([D, H], BF, name="QT")
        KTl = big.tile([D, H], BF, name="KTl")
        KTr = big.tile([D, H], BF, name="KTr")
        blobK = big.tile([P, 512], F32, name="blobK")   # K^T, folded 2x64 rows
        blobV = big.tile([P, 512], F32, name="blobV")   # VRt, 8 chunks of 64
        kvsK = big.tile([P, 512], F32, name="kvsK")
        kvsV = big.tile([P, 512], F32, name="kvsV")
        partnerK = big.tile([P, 512], F32, name="partnerK")
        partnerV = big.tile([P, 512], F32, name="partnerV")
        attnT = big.tile([P, KC, H], BF, name="attnT")
        den2 = big.tile([P, KC, 2], F32, name="den2")
        dsum = big.tile([P, KC, 2], F32, name="dsum")
        denf = big.tile([P, KC], F32, name="denf")
        r_sb = big.tile([P, KC], F32, name="r_sb")
        VRp = big.tile([P, KC, P], BF, name="VRp")
        nc.gpsimd.memset(VRp[:], 0.0)
        O1T = big.tile([DV, H], BF, name="O1T")
        WvLT = const.tile([DV, E], BF)
        bias_q = const.tile([D, 1], F32)
        bias_k = const.tile([D, 1], F32)
        bvR_b = const.tile([D, 1], BF)

        kvK_dram = dram.tile([P, 512], F32)
        kvK_sum = dram.tile([P, 512], F32)
        kvV_dram = dram.tile([P, 512], F32)
        kvV_sum = dram.tile([P, 512], F32)
        den_dram = dram.tile([P, KC * 2], F32)
        den_sum = dram.tile([P, KC * 2], F32)

        nc.gpsimd.memset(den2[:], 0.0)
        nc.gpsimd.dma_start(bvl_s[:], aps["bvL"].rearrange("(o f) -> o f", o=1))

        with tc.tile_pool(name="tp_ps", bufs=2, space="PSUM") as tp_ps, \
             tc.tile_pool(name="pj_ps", bufs=2, space="PSUM") as pj_ps, \
             tc.tile_pool(name="sc_ps", bufs=2, space="PSUM") as sc_ps:

            # weight transposes: W_s [64, E] -> WT [128, EB, 64]
            def build_wT(w_s, name):
                wt = const.tile([P, EB, D], BF, name=name)
                ps = tp_ps.tile([P, 8 * D], BF, name="tp")
                for ec in range(EB):
                    nc.tensor.transpose(ps[:, ec * D:(ec + 1) * D],
                                        w_s[:, ec * P:(ec + 1) * P],
                                        identB[0:D, 0:D])
                nc.vector.tensor_copy(
                    wt[:], ps[:].rearrange("p (a b) -> p a b", a=EB))
                return wt

            WkT = build_wT(Wk_s, "WkT")

            # biases -> per-partition columns
            bps = pj_ps.tile([P, 256], F32, name="pj")
            nc.tensor.transpose(bps[0:D, 0:3], bias_stage[0:3, :], identF[0:3, 0:3])
            nc.vector.tensor_copy(bias_q[:], bps[0:D, 0:1])
            nc.vector.tensor_copy(bias_k[:], bps[0:D, 1:2])
            nc.vector.tensor_copy(bvR_b[:], bps[0:D, 2:3])

            WqT = build_wT(Wq_s, "WqT")

            if stop_stage <= -2:
                nc.sync.dma_start(out_ap[0:P, 0:P], WqT[:, 0, :].bitcast(BF))
                return

            # ---------------- block-level helpers ----------------
            def transpose_block(src, i, dstT, acts=(0,)):
                xb = inb[(src, i)]
                for c in range(2):
                    ps = tp_ps.tile([P, 8 * P], BF, name="tp")
                    for ec in range(EB):
                        nc.tensor.transpose(ps[:, ec * P:(ec + 1) * P],
                                            xb[:, c, ec * P:(ec + 1) * P],
                                            identB[:])
                    dst = dstT[:, :, i * 256 + c * P: i * 256 + (c + 1) * P]
                    src_ps = ps[:].rearrange("p (a b) -> p a b", a=EB)
                    if c in acts:
                        nc.scalar.copy(dst, src_ps)
                    else:
                        nc.vector.tensor_copy(dst, src_ps)

            def q_chain(i):
                ps = pj_ps.tile([P, 256], F32, name="pj")
                for ec in range(EB):
                    nc.tensor.matmul(ps[0:D, :], WqT[:, ec, :],
                                     xT[:, ec, i * 256:(i + 1) * 256],
                                     start=(ec == 0), stop=(ec == EB - 1))
                nc.vector.tensor_scalar_add(QT[:, i * 256:(i + 1) * 256],
                                            ps[0:D, :], bias_q[:])

            def k_chain(i):
                ps = pj_ps.tile([P, 256], F32, name="pj")
                for ec in range(EB):
                    nc.tensor.matmul(ps[0:D, :], WkT[:, ec, :],
                                     yT[:, ec, i * 256:(i + 1) * 256],
                                     start=(ec == 0), stop=(ec == EB - 1))
                r0 = (i // 2) * D
                c0 = (i % 2) * 256
                nc.vector.tensor_scalar_add(blobK[r0:r0 + D, c0:c0 + 256],
                                            ps[0:D, :], bias_k[:])

            def cast_ktl(i):
                # blob K area block i -> KTl bf16 cols i*256..+256
                r0 = (i // 2) * D
                c0 = (i % 2) * 256
                nc.vector.tensor_copy(KTl[:, i * 256:(i + 1) * 256],
                                      blobK[r0:r0 + D, c0:c0 + 256])

            def vr_chain(kb):
                ps = pj_ps.tile([P, 256], F32, name="pj")
                for ec in range(EB):
                    nc.tensor.matmul(ps[:, 0:D], yT[:, ec, kb * P:(kb + 1) * P],
                                     WvRT[:, ec, :],
                                     start=(ec == 0), stop=(ec == EB - 1))
                nc.vector.tensor_copy(blobV[:, kb * D:(kb + 1) * D],
                                      ps[:, 0:D])

            def score_exp(kcg, qc, kt):
                # fine-grained: one q-half of one k-chunk
                sps = sc_ps.tile([P, 1024], F32, name="sc")
                kcc = kcg % 8
                nc.tensor.matmul(sps[:, 0:512], kt[:, kcc * P:(kcc + 1) * P],
                                 QT[:, qc * 512:(qc + 1) * 512],
                                 start=True, stop=True)
                nc.scalar.activation(attnT[:, kcg, qc * 512:(qc + 1) * 512],
                                     sps[:, 0:512], EXP, scale=0.125,
                                     accum_out=den2[:, kcg, qc:qc + 1])

            def score_exp2(kcg, kt):
                # merged: both q-halves of one k-chunk in one activation
                sps = sc_ps.tile([P, 1024], F32, name="sc")
                kcc = kcg % 8
                for qc in range(2):
                    nc.tensor.matmul(sps[:, qc * 512:(qc + 1) * 512],
                                     kt[:, kcc * P:(kcc + 1) * P],
                                     QT[:, qc * 512:(qc + 1) * 512],
                                     start=True, stop=True)
                nc.scalar.activation(attnT[:, kcg, :], sps[:], EXP, scale=0.125,
                                     accum_out=den2[:, kcg, 0:1])

            # ---------------- streamed main phase ----------------
            transpose_block("x", 0, xT, acts=())
            q_chain(0)
            transpose_block("x", 1, xT, acts=())
            q_chain(1)
            transpose_block("y", 0, yT, acts=())
            k_chain(0)
            cast_ktl(0)

            if stop_stage <= 1:
                nc.sync.dma_start(out_ap[0:D, 0:512], QT[:, 0:512].bitcast(BF))
                return

            score_exp(0, 0, KTl)
            score_exp(1, 0, KTl)
            transpose_block("y", 1, yT, acts=())
            k_chain(1)
            cast_ktl(1)
            score_exp(2, 0, KTl)
            score_exp(3, 0, KTl)
            WvRT = build_wT(WvR_s, "WvRT")
            vr_chain(0)
            vr_chain(1)
            transpose_block("y", 2, yT, acts=())
            k_chain(2)
            cast_ktl(2)
            score_exp(4, 0, KTl)
            score_exp(5, 0, KTl)
            vr_chain(2)
            vr_chain(3)
            transpose_block("y", 3, yT, acts=())
            k_chain(3)
            cast_ktl(3)
            score_exp(6, 0, KTl)
            score_exp(7, 0, KTl)

            # K exchange first (critical path for remote scores); VR follows
            nc.gpsimd.dma_start(kvK_dram[:], blobK[:])
            if no_cc:
                nc.gpsimd.dma_start(kvK_sum[0:P, 0:128], kvK_dram[0:P, 0:128])
                nc.gpsimd.dma_start(kvK_sum[0:P, 128:512], kvK_dram[0:P, 128:512])
            else:
                nc.gpsimd.collective_compute(
                    "AllReduce", ADD, replica_groups=GROUPS,
                    ins=[kvK_dram.opt()], outs=[kvK_sum.opt()])
            nc.gpsimd.dma_start(kvsK[:], kvK_sum[:])

            vr_chain(4)
            vr_chain(5)
            transpose_block("x", 2, xT, acts=())
            q_chain(2)
            vr_chain(6)
            vr_chain(7)

            nc.sync.dma_start(kvV_dram[:], blobV[:])
            if no_cc:
                nc.sync.dma_start(kvV_sum[:], kvV_dram[:])
            else:
                nc.gpsimd.collective_compute(
                    "AllReduce", ADD, replica_groups=GROUPS,
                    ins=[kvV_dram.opt()], outs=[kvV_sum.opt()])
            nc.sync.dma_start(kvsV[:], kvV_sum[:])

            transpose_block("x", 3, xT, acts=())
            q_chain(3)
            nc.vector.tensor_sub(partnerK[:], kvsK[:], blobK[:])
            nc.vector.tensor_copy(KTr[:, 0:512], partnerK[0:D, :])
            nc.vector.tensor_copy(KTr[:, 512:1024], partnerK[D:P, :])

            if stop_stage <= 2:
                nc.sync.dma_start(out_ap[0:D, 0:512], KTr[:, 0:512].bitcast(BF))
                return

            for kcg in range(0, 8):
                score_exp(kcg, 1, KTl)
            nc.vector.tensor_sub(partnerV[:], kvsV[:], blobV[:])
            for kcg in range(8, 16):
                score_exp2(kcg, KTr)

            # WvLT build (needed only by the finals)
            for g in range(2):
                psw = tp_ps.tile([P, 4 * P], BF, name="tp")
                for j in range(4):
                    vo = 4 * g + j
                    nc.tensor.transpose(psw[0:D, j * P:(j + 1) * P],
                                        wvls[:, vo, :], identB[:])
                nc.vector.tensor_copy(WvLT[0:D, g * 512:(g + 1) * 512],
                                      psw[0:D, :])
            for uq in range(4):
                psu = pj_ps.tile([P, 256], F32, name="pj")
                nc.tensor.matmul(
                    psu[0:1, :], bvR_b[:],
                    WvLT[0:D, uq * 256:(uq + 1) * 256],
                    start=True, stop=True)
                nc.vector.tensor_add(WvLT[D:DV, uq * 256:(uq + 1) * 256],
                                     psu[0:1, :],
                                     bvl_s[:, uq * 256:(uq + 1) * 256])

            # PE warm-up filler: keeps the tensor engine streaming through
            # the den-collective latency so O1/finals start at full clock
            wps = sc_ps.tile([P, 1024], F32, name="sc")
            for i in range(16):
                nc.tensor.matmul(wps[:, 0:512], xT[:, i % 8, 0:P],
                                 xT[:, (i + 1) % 8, 0:512],
                                 start=(i == 0), stop=(i == 15))

            # den exchange
            nc.sync.dma_start(den_dram[:], den2[:].rearrange("p a b -> p (a b)"))
            if no_cc:
                nc.sync.dma_start(den_sum[:], den_dram[:])
            else:
                nc.gpsimd.collective_compute(
                    "AllReduce", ADD, replica_groups=GROUPS,
                    ins=[den_dram.opt()], outs=[den_sum.opt()])
            nc.sync.dma_start(dsum[:].rearrange("p a b -> p (a b)"), den_sum[:])

        if stop_stage <= 3:
            nc.sync.dma_start(out_ap[0:P, 0:KC], attnT[:, :, 0].bitcast(BF))
            return

        with tc.tile_pool(name="o1_ps", bufs=2, space="PSUM") as o1_ps, \
             tc.tile_pool(name="fin_ps", bufs=3, space="PSUM") as fin_ps:

            nc.vector.tensor_reduce(denf[:], dsum[:],
                                    axis=mybir.AxisListType.X, op=ADD)
            nc.vector.reciprocal(r_sb[:], denf[:])

            # VR' = [VR * r | r | 0-pad]   (pad pre-zeroed)
            for kcg in range(KC):
                src = blobV if kcg < KCL else partnerV
                kb = kcg % KCL
                nc.vector.tensor_scalar_mul(
                    VRp[:, kcg, 0:D], src[:, kb * D:(kb + 1) * D],
                    r_sb[:, kcg:kcg + 1])
                nc.vector.tensor_copy(VRp[:, kcg, D:DV], r_sb[:, kcg:kcg + 1])

            if stop_stage <= 4:
                nc.sync.dma_start(out_ap[0:P, 0:KC * P],
                                  VRp[:].rearrange("p a b -> p (a b)").bitcast(BF))
                return

            # O1T = VR'^T @ attnT ; out = O1T^T @ [WvLT ; u]
            def o1_chain(qh):
                ops = o1_ps.tile([P, 256], F32, name="o1")
                for kcg in range(KC):
                    nc.tensor.matmul(ops[:], VRp[:, kcg, :],
                                     attnT[:, kcg, qh * 256:(qh + 1) * 256],
                                     start=(kcg == 0), stop=(kcg == KC - 1))
                nc.scalar.copy(O1T[:, qh * 256:(qh + 1) * 256], ops[0:DV, :])

            def final(qo):
                fps = fin_ps.tile([P, E], F32, name="fin")
                for vc in range(2):
                    nc.tensor.matmul(fps[:, vc * 512:(vc + 1) * 512],
                                     O1T[:, qo * P:(qo + 1) * P],
                                     WvLT[:, vc * 512:(vc + 1) * 512],
                                     start=True, stop=True)
                ob = outp.tile([P, E], BF, name="ob")
                nc.vector.tensor_copy(ob[:, 0:512], fps[:, 0:512])
                nc.scalar.copy(ob[:, 512:1024], fps[:, 512:1024])
                nc.sync.dma_start(out_ap[qo * P:(qo + 1) * P, :], ob[:])

            for qh in range(4):
                o1_chain(qh)
                final(2 * qh)
                final(2 * qh + 1)


def build_nc(reps: int = 1, no_cc=False, stop_stage=99):
    nc = bacc.Bacc("TRN2", target_bir_lowering=False, debug=False,
                   num_devices=N_CORES)
    aps = {name: nc.dram_tensor(name, shape, dt, kind="ExternalInput").ap()
           for name, shape, dt in IN_SPECS}
    out_ap = nc.dram_tensor("out", [H, E], BF, kind="ExternalOutput").ap()
    with tile.TileContext(nc) as tc:
        if reps == 1:
            _emit(tc, aps, out_ap, no_cc=no_cc, stop_stage=stop_stage)
        else:
            with tc.For_i(0, reps, 1):
                _emit(tc, aps, out_ap, no_cc=no_cc, stop_stage=stop_stage)
    nc.compile()
    return nc


def make_in_maps(inputs):
    import ml_dtypes
    bf = ml_dtypes.bfloat16
    arrs = {k: np.asarray(v) for k, v in inputs.items()}
    wb = {w: np.ascontiguousarray(arrs[w].astype(bf))
          for w in ("Wq", "Wk", "WvR", "WvL")}
    bi = {b: np.ascontiguousarray(arrs[b].astype(np.float32))
          for b in ("bq", "bk", "bvR", "bvL")}
    xb = arrs["x"].astype(bf)
    yb = arrs["y"].astype(bf)
    in_maps = []
    for c in range(N_CORES):
        b, h = divmod(c, 2)
        m = {"x": np.ascontiguousarray(xb[b, h * H:(h + 1) * H, :]),
             "y": np.ascontiguousarray(yb[b, h * H:(h + 1) * H, :])}
        m.update(wb)
        m.update(bi)
        in_maps.append(m)
    return in_maps


def assemble_out(results):
    out = np.empty((B, S, E), dtype=np.float32)
    for c in range(N_CORES):
        b, h = divmod(c, 2)
        out[b, h * H:(h + 1) * H, :] = results[c]["out"].astype(np.float32)
    return out


_NC = None


def kernel(**inputs) -> np.ndarray:
    global _NC
    if _NC is None:
        _NC = build_nc()
    in_maps = make_in_maps(inputs)
    res = run_bass_kernel_spmd(_NC, in_maps, list(range(N_CORES)))
    return assemble_out(res.results)
